# revision 12
# baseline (speedup 1.0000x reference)
"""GNN clone-detection kernel for 8 Trainium2 NeuronCores — v2.

Strategy (graph/data parallel, as before): 512 component graphs -> 64 per
core; nodes split at graph boundaries; host does integer index prep only.

v2 replaces the per-lane indirect_dma_start gathers (1 instruction per 128
rows, ~1.4 us each: SWDGE fixed overhead dominated) with batched
InstDMAGatherAnt (gpsimd.dma_gather) instructions:
 - up to 1024 rows per instruction (Q7 scratch cap), int16 indices,
   256B table rows, round-robin over 4 SWDGE queues (each queue runs on its
   own pair of Q7 cores, so descriptor generation parallelises ~4x).
 - P1 (GCN) gathers straight from a padded embedding table [V+1, 64] f32
   (embedding lookup folded into the edge gather; token ids are int16-safe).
 - P2 (SAGE) gathers from a bf16 hp table packed 4 nodes per 256B row
   (row id = tabrow//4 fits int16); a per-edge chunk id stream + is_equal
   masks select the right 32-feature slice on the vector engine before the
   per-dst lane max-reduce.

Measured piecewise: dma_gather sustains ~2.9 ns/row at 4 queues vs ~11 ns/row
for the baseline's per-lane indirect DMAs (both Q7 descriptor-generation
bound; 1024 idxs/instruction is a hard Q7 scratch cap).  The P2 chunk-select
runs as one fused mask + one masked multiply + a pairwise-max tree so the
DVE stays under the Pool gather time.

Measured: 4.81 ms HW exec on 8 cores (P1 2.35 ms + P2 2.47 ms), rel err
8.3e-7 vs the jax reference; baseline was 18.16 ms.
"""

import sys
import types

sys.path.insert(0, '/opt/trn_rl_repo')

import numpy as np

# ---------------------------------------------------------------------------
# Environment shims (this container's walrus encodes at most ONE sync wait
# per instruction; split extra waits onto NoOps).  Also provide the missing
# antenv.axon_hooks module so bass_utils imports cleanly under axon.
# ---------------------------------------------------------------------------
import antenv  # noqa: E402

if 'antenv.axon_hooks' not in sys.modules:
    _hooks = types.ModuleType('antenv.axon_hooks')
    _hooks._hook = None

    def _set_hook(h):
        _hooks._hook = h

    def _get_hook():
        if _hooks._hook is None:
            try:
                from trn_agent_boot.trn_boot import _ntff_profile_via_ctypes
                _hooks._hook = _ntff_profile_via_ctypes('/opt/axon/libaxon_pjrt.so')
            except Exception:
                return None
        return _hooks._hook

    _hooks.set_axon_ntff_profile_hook = _set_hook
    _hooks.get_axon_ntff_profile_hook = _get_hook
    sys.modules['antenv.axon_hooks'] = _hooks
    antenv.axon_hooks = _hooks

import concourse.bass as bass  # noqa: E402
import concourse.mybir as mybir  # noqa: E402
import concourse.tile as tile  # noqa: E402
from concourse.vector_clock import ScopedClock  # noqa: E402
from concourse.bass_utils import run_bass_kernel_spmd  # noqa: E402
from concourse.library_config import mlp as _mlp_lib  # noqa: E402
from concourse.library_overlay import lower_extended_insts  # noqa: E402

_split_counter = [0]


def _emit_split_nops(nc, inst, add):
    si = inst.sync_info
    if si is not None and si.on_wait is not None and len(si.on_wait) > 1:
        waits = list(si.on_wait)
        si.on_wait = [waits[-1]]
        for w in waits[:-1]:
            _split_counter[0] += 1
            nop = mybir.InstNoOp(
                name=f"splitw-{_split_counter[0]}",
                engine=inst.engine,
                sync_info=mybir.SyncInfo(on_wait=[w], on_update=[]),
                bass_nofuse=True,
            )
            add(nop)


if not getattr(tile.TileContext, '_gnn_patched', False):
    _orig_add_instruction = tile.TileContext._add_instruction

    def _patched_add_instruction(self, inst):
        def add(i):
            self.nc.register_instruction(i, overwrite=True)
            self.nc.cur_bb.bb.add_instruction(i)

        _emit_split_nops(self.nc, inst, add)
        _orig_add_instruction(self, inst)

    def _patched_drain_and_barrier(self, tick_clock, wait_clock):
        nc = self.nc
        drain_inst = nc.sync.drain()
        wait_clock.add_sem_waits(
            drain_inst.ins, ScopedClock({None: tick_clock.global_clock})
        )
        si = drain_inst.ins.sync_info
        if si is not None and si.on_wait is not None and len(si.on_wait) > 1:
            waits = list(si.on_wait)
            si.on_wait = waits[:1]
            for w in waits[1:]:
                nop = nc.sync.nop(nofuse=True)
                nsi = nop.ins.sync_info
                if nsi is None:
                    nop.ins.sync_info = mybir.SyncInfo(on_wait=[w], on_update=[])
                else:
                    nsi.on_wait = [w]
        nc.all_engine_barrier()
        assert self.sems is not None
        popped = nc._tile_sem_poison_stack.pop()
        assert popped is self._sem_poison
        nc.clear_and_free_semaphores(list(self.sems.allocated().values()))
        nc.all_engine_barrier()

    tile.TileContext._add_instruction = _patched_add_instruction
    tile.TileContext._drain_and_barrier = _patched_drain_and_barrier
    tile.TileContext._gnn_patched = True

# ---------------------------------------------------------------------------
# Problem constants (hardcoded per the task contract).
# ---------------------------------------------------------------------------
N = 100000
E = 3200000
G = 512
V = 8018
NC = 8
GPC = G // NC           # graphs per core
P = 128
F32 = mybir.dt.float32
BF16 = mybir.dt.bfloat16
I16 = mybir.dt.int16
NQ = 4                  # SWDGE queues

_CORES = list(range(NC))


def _host_prep_side(tokens, src, dst, nid):
    deg_out = np.bincount(src, minlength=N).astype(np.int64)
    deg_in = np.bincount(dst, minlength=N).astype(np.int64)

    gcounts = np.bincount(nid, minlength=G).astype(np.int64)
    gstart = np.zeros(G + 1, np.int64)
    np.cumsum(gcounts, out=gstart[1:])
    node_lo = np.array([gstart[c * GPC] for c in range(NC)] + [N])

    cores = []
    for c in range(NC):
        lo, hi = int(node_lo[c]), int(node_lo[c + 1])
        nodes = np.arange(lo, hi)
        order = np.argsort(-deg_in[nodes], kind='stable')
        perm = nodes[order]
        cores.append(dict(lo=lo, hi=hi, perm=perm,
                          deg_in=deg_in[perm], nid_local=nid[perm] - c * GPC))
    return dict(deg_out=deg_out, deg_in=deg_in, src=src, dst=dst,
                tokens=tokens, cores=cores)


def _lane_groups(L):
    """Split L lanes into dma_gather groups of <= 8 lanes (1024 idxs)."""
    gs = [8] * (L // 8)
    if L % 8:
        gs.append(L % 8)
    return gs


def _build_csr2(side, nodes_pad, zero_row):
    """Per-side CSR in two layouts:
     - lane-major entry order (for gather idx arrays, wrapped int16)
     - p-major order (for dgo / chunk-id streams)
    L[t] is the per-tile lane count, max over the 8 cores."""
    ntiles = nodes_pad // P
    tabrow = np.empty(N, np.int64)
    for c, info in enumerate(side['cores']):
        tabrow[info['perm']] = c * nodes_pad + np.arange(len(info['perm']))

    # per-tile max in-degree over cores
    L = np.zeros(ntiles, np.int64)
    for info in side['cores']:
        d = np.zeros(nodes_pad, np.int64)
        d[:len(info['deg_in'])] = info['deg_in']
        L = np.maximum(L, d.reshape(ntiles, P).max(axis=1))
    L = np.maximum(L, 1)
    co = np.zeros(ntiles + 1, np.int64)
    np.cumsum(L, out=co[1:])
    tot = int(co[-1])

    src, dst = side['src'], side['dst']
    tokens, deg_out = side['tokens'], side['deg_out']
    srow = tabrow[src]

    tokf = np.full((NC, tot * P), V, np.int16)          # pad -> zero embed row
    hpf = np.full((NC, tot * P), zero_row, np.int16)    # pad -> zero hp row
    dgof = np.ones((NC, tot * P), np.float32)
    chf = np.zeros((NC, tot * P), np.float32)
    for c, info in enumerate(side['cores']):
        lo, hi = info['lo'], info['hi']
        m = (dst >= lo) & (dst < hi)
        erow = tabrow[dst[m]] - c * nodes_pad
        es = src[m]
        esr = srow[m]
        order = np.argsort(erow, kind='stable')
        erow = erow[order]
        es = es[order]
        esr = esr[order]
        counts = np.bincount(erow, minlength=nodes_pad)
        starts = np.zeros(nodes_pad, np.int64)
        np.cumsum(counts[:-1], out=starts[1:])
        lane = np.arange(len(erow)) - starts[erow]
        t = erow // P
        p = erow % P
        posL = (co[t] + lane) * P + p          # lane-major within tile
        posP = co[t] * P + p * L[t] + lane     # p-major within tile
        tokf[c, posL] = tokens[es].astype(np.int16)
        hpf[c, posL] = (esr // 4).astype(np.int16)
        dgof[c, posP] = deg_out[es].astype(np.float32)
        chf[c, posP] = (esr % 4).astype(np.float32)

    # wrap lane-major idx arrays into per-instruction [16, 8k] blocks
    def wrap(flat):
        blocks = []
        for t in range(ntiles):
            b0 = int(co[t]) * P
            l0 = 0
            for k in _lane_groups(int(L[t])):
                v = flat[:, b0 + l0 * P: b0 + (l0 + k) * P]   # [NC, 128k]
                blocks.append(v.reshape(NC, 8 * k, 16).transpose(0, 2, 1))
                l0 += k
        w16 = np.concatenate(blocks, axis=2)                  # [NC, 16, 8*tot]
        return np.tile(w16, (1, 8, 1))                        # replicate cores

    return dict(L=L.astype(int), co=co, tot=tot,
                tokw=wrap(tokf), hpw=wrap(hpf), dgo=dgof, ch=chf)


def _pack_params(inputs):
    pr = {}
    for k in ('embed', 'gcn1_W', 'gcn1_b', 'pool_W', 'pool_b', 'self_W',
              'neigh_W', 'sage_b', 'lg_W', 'lg_b', 'top_W', 'top_b'):
        pr[k] = np.asarray(inputs[k], np.float32)
    return pr


# ---------------------------------------------------------------------------
# Device programs
# ---------------------------------------------------------------------------

def _prog_gcn(nodes_pad, Ls, cos, tots):
    """P1: GCN pass; embedding lookup folded into batched dma_gather.
    inputs: embp [V+2? no: V+1 padded to 64 cols] -> [V+1, 64] f32 (row V = 0)
            tokw{s} [128, 8*tot_s] i16; dgo{s} [tot_s*128, 1] f32;
            din{s} [nodes_pad,1] f32; wg [16,32]; bg [32,1]; wp [32,32]; bp [32,1]
    outputs: h1t{s} [32, nodes_pad] f32; hp{s} [nodes_pad, 32] bf16"""
    nc = bass.Bass(target_bir_lowering=False, num_swdge_queues=NQ)
    tokws, dgos, dins, h1ts, hps = [], [], [], [], []
    for s in (1, 2):
        tot = tots[s - 1]
        tokws.append(nc.dram_tensor(f"tokw{s}", [128, 8 * tot], I16, kind="ExternalInput"))
        dgos.append(nc.dram_tensor(f"dgo{s}", [tot * P, 1], F32, kind="ExternalInput"))
        dins.append(nc.dram_tensor(f"din{s}", [nodes_pad, 1], F32, kind="ExternalInput"))
        h1ts.append(nc.dram_tensor(f"h1t{s}", [32, nodes_pad], F32, kind="ExternalOutput"))
        hps.append(nc.dram_tensor(f"hp{s}", [nodes_pad, 32], BF16, kind="ExternalOutput"))
    embp = nc.dram_tensor("embp", [V + 1, 64], F32, kind="ExternalInput")
    wg = nc.dram_tensor("wg", [16, 32], F32, kind="ExternalInput")
    bg = nc.dram_tensor("bg", [32, 1], F32, kind="ExternalInput")
    wp = nc.dram_tensor("wp", [32, 32], F32, kind="ExternalInput")
    bp = nc.dram_tensor("bp", [32, 1], F32, kind="ExternalInput")

    ntiles = nodes_pad // P
    from concourse.masks import make_identity
    qrr = [0]
    with tile.TileContext(nc) as tc:
        with tc.tile_pool(name="const", bufs=1) as cpool, \
             tc.tile_pool(name="sb", bufs=3) as pool, \
             tc.tile_pool(name="ps", bufs=2, space="PSUM") as psp:
            ident = cpool.tile([P, P], F32)
            make_identity(nc, ident[:])
            nc.gpsimd.load_library(_mlp_lib)
            regs = {k: nc.gpsimd.to_reg(128 * k) for k in range(1, 9)}
            wg_sb = cpool.tile([16, 32], F32)
            nc.sync.dma_start(wg_sb[:], wg[:, :])
            bg_sb = cpool.tile([32, 1], F32)
            nc.sync.dma_start(bg_sb[:], bg[:, :])
            wp_sb = cpool.tile([32, 32], F32)
            nc.sync.dma_start(wp_sb[:], wp[:, :])
            bp_sb = cpool.tile([32, 1], F32)
            nc.sync.dma_start(bp_sb[:], bp[:, :])

            for s in range(2):
                L, co = Ls[s], cos[s]
                for t in range(ntiles):
                    Lt = int(L[t])
                    it = pool.tile([P, 8 * Lt], I16, tag="it", bufs=4)
                    nc.sync.dma_start(
                        it[:], tokws[s][:, 8 * int(co[t]):8 * int(co[t] + Lt)])
                    dg = pool.tile([P, Lt], F32, tag="dg", bufs=4)
                    nc.sync.dma_start(
                        dg[:], dgos[s][int(co[t]) * P:int(co[t] + Lt) * P, 0]
                        .rearrange("(p l) -> p l", l=Lt))
                    g = pool.tile([P, Lt * 64], F32, tag="g", bufs=3)
                    l0 = 0
                    for k in _lane_groups(Lt):
                        nc.gpsimd.dma_gather(
                            g[:, l0 * 64:(l0 + k) * 64]
                            .rearrange("p (j e) -> p j e", e=64),
                            embp[:, :], it[:, 8 * l0:8 * (l0 + k)],
                            128 * k, regs[k], 64, queue_num=qrr[0] % NQ)
                        qrr[0] += 1
                        l0 += k
                    # per-lane deg_out^-1/2 scale
                    dgm = pool.tile([P, Lt], F32, tag="dgm")
                    nc.vector.tensor_scalar_max(dgm[:], dg[:], 1.0)
                    dgs = pool.tile([P, Lt], F32, tag="dgs")
                    nc.scalar.activation(dgs[:], dgm[:],
                                         mybir.ActivationFunctionType.Sqrt)
                    dgr = pool.tile([P, Lt], F32, tag="dgr")
                    nc.vector.reciprocal(dgr[:], dgs[:])
                    g2 = pool.tile([P, Lt * 16], F32, tag="g2")
                    nc.vector.tensor_tensor(
                        out=g2[:].rearrange("p (l f) -> p l f", l=Lt, f=16),
                        in0=g[:].rearrange("p (l e) -> p l e", l=Lt, e=64)[:, :, 0:16],
                        in1=dgr[:].rearrange("p (l o) -> p l o", o=1)
                        .to_broadcast([P, Lt, 16]),
                        op=mybir.AluOpType.mult)
                    m = pool.tile([P, 16], F32, tag="m")
                    nc.vector.tensor_reduce(
                        m[:], g2[:].rearrange("p (l f) -> p f l", l=Lt, f=16),
                        axis=mybir.AxisListType.X, op=mybir.AluOpType.add)
                    ds = pool.tile([P, 1], F32, tag="ds")
                    nc.sync.dma_start(ds[:], dins[s][t * P:(t + 1) * P, :])
                    dm = pool.tile([P, 1], F32, tag="dm")
                    nc.vector.tensor_scalar_max(dm[:], ds[:], 1.0)
                    sq = pool.tile([P, 1], F32, tag="sq")
                    nc.scalar.activation(sq[:], dm[:], mybir.ActivationFunctionType.Sqrt)
                    rc = pool.tile([P, 1], F32, tag="rc")
                    nc.vector.reciprocal(rc[:], sq[:])
                    ms = pool.tile([P, 16], F32, tag="ms")
                    nc.vector.tensor_tensor(out=ms[:], in0=m[:],
                                            in1=rc[:].to_broadcast([P, 16]),
                                            op=mybir.AluOpType.mult)
                    mt_ps = psp.tile([16, P], F32, tag="mt", space="PSUM")
                    nc.tensor.transpose(out=mt_ps[:], in_=ms[:], identity=ident[:])
                    mt = pool.tile([16, P], F32, tag="mt_sb")
                    nc.scalar.copy(mt[:], mt_ps[:])
                    h1_ps = psp.tile([32, P], F32, tag="h1", space="PSUM")
                    nc.tensor.matmul(h1_ps[:], lhsT=wg_sb[:], rhs=mt[:],
                                     start=True, stop=True)
                    h1 = pool.tile([32, P], F32, tag="h1sb")
                    nc.scalar.activation(h1[:], h1_ps[:],
                                         mybir.ActivationFunctionType.Relu,
                                         bias=bg_sb[:])
                    nc.sync.dma_start(h1ts[s][:, t * P:(t + 1) * P], h1[:])
                    hp_ps = psp.tile([32, P], F32, tag="hp", space="PSUM")
                    nc.tensor.matmul(hp_ps[:], lhsT=wp_sb[:], rhs=h1[:],
                                     start=True, stop=True)
                    hpT = pool.tile([32, P], F32, tag="hpT")
                    nc.scalar.activation(hpT[:], hp_ps[:],
                                         mybir.ActivationFunctionType.Relu,
                                         bias=bp_sb[:])
                    hpn_ps = psp.tile([P, 32], F32, tag="hpn", space="PSUM")
                    nc.tensor.transpose(out=hpn_ps[:], in_=hpT[:],
                                        identity=ident[:32, :32])
                    hpn = pool.tile([P, 32], BF16, tag="hpn_sb")
                    nc.vector.tensor_copy(hpn[:], hpn_ps[:])
                    nc.sync.dma_start(hps[s][t * P:(t + 1) * P, :], hpn[:])
    lower_extended_insts(nc)
    return nc


def _prog_sage(nodes_pad, R2, Ls, cos, tots):
    """P2: SAGE pass + readout + top MLP + cosine.
    inputs: hpt{s} [R2, 128] bf16 (packed hp table, 4 nodes/row, last row 0);
            hpw{s} [128, 8*tot_s] i16; ch{s} [tot_s*128, 1] f32;
            h1t{s} [32, nodes_pad]; nl{s} [nodes_pad,1] f32;
            ws [32,64]; wn [32,64]; bs [64,1]; wlb [65,64]; wt [64,128];
            bt [128,1]; iota64 [P,64]
    outputs: sim [1, 64]"""
    nc = bass.Bass(target_bir_lowering=False, num_swdge_queues=NQ)
    hpts, hpws, chs, h1ts, nls = [], [], [], [], []
    for s in (1, 2):
        tot = tots[s - 1]
        hpts.append(nc.dram_tensor(f"hpt{s}", [R2, 128], BF16, kind="ExternalInput"))
        hpws.append(nc.dram_tensor(f"hpw{s}", [128, 8 * tot], I16, kind="ExternalInput"))
        chs.append(nc.dram_tensor(f"ch{s}", [tot * P, 1], F32, kind="ExternalInput"))
        h1ts.append(nc.dram_tensor(f"h1t{s}", [32, nodes_pad], F32, kind="ExternalInput"))
        nls.append(nc.dram_tensor(f"nl{s}", [nodes_pad, 1], F32, kind="ExternalInput"))
    ws = nc.dram_tensor("ws", [32, 64], F32, kind="ExternalInput")
    wn = nc.dram_tensor("wn", [32, 64], F32, kind="ExternalInput")
    bs = nc.dram_tensor("bs", [64, 1], F32, kind="ExternalInput")
    wlb = nc.dram_tensor("wlb", [65, 64], F32, kind="ExternalInput")
    wt = nc.dram_tensor("wt", [64, 128], F32, kind="ExternalInput")
    bt = nc.dram_tensor("bt", [128, 1], F32, kind="ExternalInput")
    iot = nc.dram_tensor("iota64", [P, 64], F32, kind="ExternalInput")
    iot4 = nc.dram_tensor("iota4", [P, 4], F32, kind="ExternalInput")
    sim_o = nc.dram_tensor("sim", [1, 64], F32, kind="ExternalOutput")

    ntiles = nodes_pad // P
    from concourse.masks import make_identity
    qrr = [0]
    with tile.TileContext(nc) as tc:
        with tc.tile_pool(name="const", bufs=1) as cpool, \
             tc.tile_pool(name="sb", bufs=3) as pool, \
             tc.tile_pool(name="acc", bufs=1) as accp, \
             tc.tile_pool(name="ps", bufs=1, space="PSUM") as psp:
            ident = cpool.tile([P, P], F32)
            make_identity(nc, ident[:])
            ws_sb = cpool.tile([32, 64], F32)
            nc.sync.dma_start(ws_sb[:], ws[:, :])
            wn_sb = cpool.tile([32, 64], F32)
            nc.sync.dma_start(wn_sb[:], wn[:, :])
            bs_sb = cpool.tile([64, 1], F32)
            nc.sync.dma_start(bs_sb[:], bs[:, :])
            wlb_sb = cpool.tile([65, 64], F32)
            nc.sync.dma_start(wlb_sb[:], wlb[:, :])
            wt_sb = cpool.tile([64, P], F32)
            nc.sync.dma_start(wt_sb[:], wt[:, :])
            bt_sb = cpool.tile([P, 1], F32)
            nc.sync.dma_start(bt_sb[:], bt[:, :])
            iota_sb = cpool.tile([P, 64], F32)
            nc.sync.dma_start(iota_sb[:], iot[:, :])
            iota4_sb = cpool.tile([P, 4], F32)
            nc.sync.dma_start(iota4_sb[:], iot4[:, :])
            ones_sb = cpool.tile([P, 1], F32)
            nc.gpsimd.memset(ones_sb[:], 1.0)

            r_sb = [accp.tile([64, 64], F32, tag=f"r{s}", name=f"racc{s}")
                    for s in range(2)]
            for s in range(2):
                nc.gpsimd.memset(r_sb[s][:], 0.0)
            nc.gpsimd.load_library(_mlp_lib)
            regs = {k: nc.gpsimd.to_reg(128 * k) for k in range(1, 9)}

            for s in range(2):
                L, co = Ls[s], cos[s]
                for t in range(ntiles):
                    Lt = int(L[t])
                    it = pool.tile([P, 8 * Lt], I16, tag="it", bufs=4)
                    nc.sync.dma_start(
                        it[:], hpws[s][:, 8 * int(co[t]):8 * int(co[t] + Lt)])
                    chl = pool.tile([P, Lt], F32, tag="chl", bufs=4)
                    nc.sync.dma_start(
                        chl[:], chs[s][int(co[t]) * P:int(co[t] + Lt) * P, 0]
                        .rearrange("(p l) -> p l", l=Lt))
                    g = pool.tile([P, Lt * 128], BF16, tag="g", bufs=3)
                    l0 = 0
                    for k in _lane_groups(Lt):
                        nc.gpsimd.dma_gather(
                            g[:, l0 * 128:(l0 + k) * 128]
                            .rearrange("p (j e) -> p j e", e=128),
                            hpts[s][:, :], it[:, 8 * l0:8 * (l0 + k)],
                            128 * k, regs[k], 128, queue_num=qrr[0] % NQ)
                        qrr[0] += 1
                        l0 += k
                    # chunk-select: one fused mask, one masked mult, then a
                    # pairwise-max tree (avoids the big strided 4L reduce)
                    mk4 = pool.tile([P, Lt * 4], BF16, tag="mk4")
                    nc.vector.tensor_tensor(
                        out=mk4[:].rearrange("p (l c) -> p l c", c=4),
                        in0=chl[:].rearrange("p (l o) -> p l o", o=1)
                        .to_broadcast([P, Lt, 4]),
                        in1=iota4_sb[:].rearrange("p (q c) -> p q c", q=1)
                        .to_broadcast([P, Lt, 4]),
                        op=mybir.AluOpType.is_equal)
                    tb = pool.tile([P, Lt * 128], BF16, tag="tb", bufs=3)
                    nc.vector.tensor_tensor(
                        out=tb[:].rearrange("p (lc f) -> p lc f", f=32),
                        in0=g[:].rearrange("p (lc f) -> p lc f", f=32),
                        in1=mk4[:].rearrange("p (lc o) -> p lc o", o=1)
                        .to_broadcast([P, 4 * Lt, 32]),
                        op=mybir.AluOpType.mult)
                    tb3 = tb[:].rearrange("p (l q) -> p l q", q=128)
                    u = pool.tile([P, Lt * 32], BF16, tag="u")
                    nc.vector.tensor_tensor(
                        out=u[:].rearrange("p (l f) -> p l f", f=32),
                        in0=tb3[:, :, 0:32], in1=tb3[:, :, 32:64],
                        op=mybir.AluOpType.max)
                    v = pool.tile([P, Lt * 32], BF16, tag="v")
                    nc.vector.tensor_tensor(
                        out=v[:].rearrange("p (l f) -> p l f", f=32),
                        in0=tb3[:, :, 64:96], in1=tb3[:, :, 96:128],
                        op=mybir.AluOpType.max)
                    w = pool.tile([P, Lt * 32], BF16, tag="w")
                    nc.vector.tensor_tensor(out=w[:], in0=u[:], in1=v[:],
                                            op=mybir.AluOpType.max)
                    nb = pool.tile([P, 32], F32, tag="nb")
                    nc.vector.tensor_reduce(
                        nb[:], w[:].rearrange("p (l f) -> p f l", f=32),
                        axis=mybir.AxisListType.X, op=mybir.AluOpType.max)
                    nt_ps = psp.tile([32, P], F32, tag="nt", space="PSUM")
                    nc.tensor.transpose(out=nt_ps[:], in_=nb[:], identity=ident[:])
                    ntb = pool.tile([32, P], F32, tag="ntb")
                    nc.scalar.copy(ntb[:], nt_ps[:])
                    h1 = pool.tile([32, P], F32, tag="h1")
                    nc.sync.dma_start(h1[:], h1ts[s][:, t * P:(t + 1) * P])
                    h2_ps = psp.tile([64, P], F32, tag="h2", space="PSUM")
                    nc.tensor.matmul(h2_ps[:], lhsT=ws_sb[:], rhs=h1[:],
                                     start=True, stop=False)
                    nc.tensor.matmul(h2_ps[:], lhsT=wn_sb[:], rhs=ntb[:],
                                     start=False, stop=True)
                    h2 = pool.tile([65, P], F32, tag="h2sb")
                    nc.scalar.activation(h2[:64, :], h2_ps[:],
                                         mybir.ActivationFunctionType.Relu,
                                         bias=bs_sb[:])
                    nc.vector.tensor_copy(h2[64:65, :],
                                          ones_sb[:1, :].to_broadcast([1, P]))
                    h3_ps = psp.tile([P, 64], F32, tag="h3", space="PSUM")
                    nc.tensor.matmul(h3_ps[:], lhsT=h2[:], rhs=wlb_sb[:],
                                     start=True, stop=True)
                    h3 = pool.tile([P, 64], F32, tag="h3sb")
                    nc.scalar.activation(h3[:], h3_ps[:],
                                         mybir.ActivationFunctionType.Relu)
                    nl = pool.tile([P, 1], F32, tag="nl")
                    nc.sync.dma_start(nl[:], nls[s][t * P:(t + 1) * P, :])
                    sel = pool.tile([P, 64], F32, tag="sel")
                    nc.vector.tensor_tensor(out=sel[:], in0=iota_sb[:],
                                            in1=nl[:].to_broadcast([P, 64]),
                                            op=mybir.AluOpType.is_equal)
                    rt_ps = psp.tile([64, 64], F32, tag="rt", space="PSUM")
                    nc.tensor.matmul(rt_ps[:], lhsT=sel[:], rhs=h3[:],
                                     start=True, stop=True)
                    nc.vector.tensor_tensor(out=r_sb[s][:], in0=r_sb[s][:],
                                            in1=rt_ps[:],
                                            op=mybir.AluOpType.add)

            # top MLP + cosine
            bT = []
            for s in range(2):
                rt2_ps = psp.tile([64, 64], F32, tag="rt2", space="PSUM")
                nc.tensor.transpose(out=rt2_ps[:], in_=r_sb[s][:],
                                    identity=ident[:64, :64])
                rt2 = pool.tile([64, 64], F32, tag=f"rt2sb{s}")
                nc.scalar.copy(rt2[:], rt2_ps[:])
                b_ps = psp.tile([P, 64], F32, tag="b", space="PSUM")
                nc.tensor.matmul(b_ps[:], lhsT=wt_sb[:], rhs=rt2[:],
                                 start=True, stop=True)
                bsb = pool.tile([P, 64], F32, tag=f"bsb{s}")
                nc.scalar.activation(bsb[:], b_ps[:],
                                     mybir.ActivationFunctionType.Relu,
                                     bias=bt_sb[:])
                bT.append(bsb)
            prods = pool.tile([P, 192], F32, tag="prods")
            nc.vector.tensor_tensor(out=prods[:, 0:64], in0=bT[0][:],
                                    in1=bT[1][:], op=mybir.AluOpType.mult)
            nc.vector.tensor_tensor(out=prods[:, 64:128], in0=bT[0][:],
                                    in1=bT[0][:], op=mybir.AluOpType.mult)
            nc.vector.tensor_tensor(out=prods[:, 128:192], in0=bT[1][:],
                                    in1=bT[1][:], op=mybir.AluOpType.mult)
            dots_ps = psp.tile([1, 192], F32, tag="dots", space="PSUM")
            nc.tensor.matmul(dots_ps[:], lhsT=ones_sb[:], rhs=prods[:],
                             start=True, stop=True)
            dots = pool.tile([1, 192], F32, tag="dots_sb")
            nc.vector.tensor_copy(dots[:], dots_ps[:])
            s1 = pool.tile([1, 128], F32, tag="s1")
            nc.scalar.activation(s1[:], dots[:, 64:192],
                                 mybir.ActivationFunctionType.Sqrt)
            s1m = pool.tile([1, 128], F32, tag="s1m")
            nc.vector.tensor_scalar_max(s1m[:], s1[:], 1e-8)
            den = pool.tile([1, 64], F32, tag="den")
            nc.vector.tensor_tensor(out=den[:], in0=s1m[:, 0:64],
                                    in1=s1m[:, 64:128], op=mybir.AluOpType.mult)
            rec = pool.tile([1, 64], F32, tag="rec")
            nc.vector.reciprocal(rec[:], den[:])
            sim = pool.tile([1, 64], F32, tag="sim")
            nc.vector.tensor_tensor(out=sim[:], in0=dots[:, 0:64], in1=rec[:],
                                    op=mybir.AluOpType.mult)
            sima = pool.tile([1, 64], F32, tag="sima")
            nc.scalar.activation(sima[:], sim[:],
                                 mybir.ActivationFunctionType.Abs)
            simc = pool.tile([1, 64], F32, tag="simc")
            nc.vector.tensor_scalar_min(simc[:], sima[:], 1.0)
            nc.sync.dma_start(sim_o[:, :], simc[:])
    lower_extended_insts(nc)
    return nc


# ---------------------------------------------------------------------------
# Orchestration
# ---------------------------------------------------------------------------

def _run(nc, in_maps, trace=False):
    return run_bass_kernel_spmd(nc, in_maps, core_ids=_CORES, trace=trace)


def kernel(_trace=False, _collect=None, **inputs) -> np.ndarray:
    import ml_dtypes
    pr = _pack_params(inputs)
    sides = []
    for s in (1, 2):
        sides.append(_host_prep_side(
            np.asarray(inputs[f'tokens{s}']).astype(np.int64),
            np.asarray(inputs[f'src{s}']).astype(np.int64),
            np.asarray(inputs[f'dst{s}']).astype(np.int64),
            np.asarray(inputs[f'nid{s}']).astype(np.int64)))

    nodes_max = max(len(info['perm']) for sd in sides for info in sd['cores'])
    nodes_pad = ((nodes_max + P - 1) // P) * P
    trows = NC * nodes_pad
    assert trows % 4 == 0
    R2 = trows // 4 + 1
    assert R2 - 1 <= 32767

    csr = [_build_csr2(sd, nodes_pad, R2 - 1) for sd in sides]
    Ls = [c['L'] for c in csr]
    cos = [c['co'] for c in csr]
    tots = [c['tot'] for c in csr]

    # per-core host arrays
    din_rows, nl_rows = [], []
    for s, sd in enumerate(sides):
        di = np.zeros((NC, nodes_pad, 1), np.float32)
        nl = np.full((NC, nodes_pad, 1), 64.0, np.float32)
        for c, info in enumerate(sd['cores']):
            k = len(info['perm'])
            di[c, :k, 0] = info['deg_in']
            nl[c, :k, 0] = info['nid_local']
        din_rows.append(di)
        nl_rows.append(nl)

    exec_ns = []

    # ---- P1: GCN ----
    nc1 = _prog_gcn(nodes_pad, Ls, cos, tots)
    embp = np.zeros((V + 1, 64), np.float32)
    embp[:V, :16] = pr['embed']
    wg = pr['gcn1_W'].T.copy()
    wp = pr['pool_W'].T.copy()
    in1 = [{"embp": embp,
            "tokw1": csr[0]['tokw'][c], "tokw2": csr[1]['tokw'][c],
            "dgo1": csr[0]['dgo'][c][:, None], "dgo2": csr[1]['dgo'][c][:, None],
            "din1": din_rows[0][c], "din2": din_rows[1][c],
            "wg": wg, "bg": pr['gcn1_b'][:, None].copy(),
            "wp": wp, "bp": pr['pool_b'][:, None].copy()} for c in range(NC)]
    r1 = _run(nc1, in1, trace=_trace)
    exec_ns.append(r1.exec_time_ns)

    # assemble packed bf16 hp tables [R2, 128]
    hpt = []
    for si, s in enumerate((1, 2)):
        full = np.concatenate([r1.results[c][f"hp{s}"] for c in range(NC)],
                              axis=0)                       # [trows, 32] bf16
        packed = np.ascontiguousarray(full).reshape(R2 - 1, 128)
        packed = np.concatenate(
            [packed, np.zeros((1, 128), ml_dtypes.bfloat16)], axis=0)
        hpt.append(packed)
    h1t = [[r1.results[c][f"h1t{s}"] for c in range(NC)] for s in (1, 2)]

    # ---- P2: SAGE + readout + cosine ----
    nc2 = _prog_sage(nodes_pad, R2, Ls, cos, tots)
    wlb = np.concatenate([pr['lg_W'].T, pr['lg_b'][None, :]], axis=0)
    iota64 = np.tile(np.arange(64, dtype=np.float32)[None, :], (P, 1))
    iota4 = np.tile(np.arange(4, dtype=np.float32)[None, :], (P, 1))
    in2 = [{"hpt1": hpt[0], "hpt2": hpt[1],
            "hpw1": csr[0]['hpw'][c], "hpw2": csr[1]['hpw'][c],
            "ch1": csr[0]['ch'][c][:, None], "ch2": csr[1]['ch'][c][:, None],
            "h1t1": h1t[0][c], "h1t2": h1t[1][c],
            "nl1": nl_rows[0][c], "nl2": nl_rows[1][c],
            "ws": pr['self_W'].T.copy(), "wn": pr['neigh_W'].T.copy(),
            "bs": pr['sage_b'][:, None].copy(), "wlb": wlb,
            "wt": pr['top_W'].T.copy(), "bt": pr['top_b'][:, None].copy(),
            "iota64": iota64, "iota4": iota4} for c in range(NC)]
    r2 = _run(nc2, in2, trace=_trace)
    exec_ns.append(r2.exec_time_ns)

    out = np.concatenate([r2.results[c]["sim"][0] for c in range(NC)])
    if _collect is not None:
        _collect['exec_ns'] = exec_ns
    return out.astype(np.float32)


if __name__ == "__main__":
    print("kernel module loaded; run test.py")
